# revision 1
# baseline (speedup 1.0000x reference)
"""Trainium2 Bass kernel for nn_MultiAgentsSummarizer (pointer-generator style
multi-agent summarizer distribution).

Math (per batch b, with T=64 target positions, A=4 agents, S=512 source tokens,
V=32000 vocab, EXT_V=33000 extended vocab):

    coef[t]   = sum_a agent_attn[t,a] * gen[t,a]
    out[t,v]  = coef[t] * vocab_probs[t,v]            (v <  V;  0 for v >= V)
    out[t, article[a,s]] += agent_attn[t,a]*(1-gen[t,a]) * agentwise_attn[t,a,s]

Strategy: one batch element per NeuronCore (B=8 = n_cores). Device work runs in
a v-major ("transposed") staging layout out[v, t]: each scatter destination is
one contiguous 256-byte DRAM row, served by GPSIMD dma_scatter_add (CCE add).
The host only reorders/relabels data (transposes, column permutation, index
tables) -- all floating-point arithmetic runs on device.

Staging is out_main [32768, 64] (v < 32768; 32768 = 256*128, and 32000 = 125*256
so the scaled-vocab base stream is exactly partitions 0..124 with no ragged
edge) plus out_hi [256, 64] (v - 32768). Rows >= 32000 have zero base and rely
on the PJRT-donated pre-zeroed output buffers.

dma_scatter_add RMW races on duplicate rows inside one call, so each call gets
one payload slot per unique destination row. Contribution layout (128-slot
chunks; payload of slot k lives at items[k % 128, (k//128)*T:...]):
  chunks 0..15  A-call singles, per-agent static ranges (tensor_scalar by c4)
  chunks 16,17  A-call merge slots M (duplicate groups, mixed agents)
  chunk  18     H-call (v >= 32768), mixed agents
  chunks 19,20  S1: rank-1 partners of M groups (same partition as the group)
  chunk  21     S2: rank-2 partners (partition-aligned with M chunk 16)
  chunk  22     S3: rank-1 partners of H rows (partition-aligned with H)
  chunk  23     S4: rank-3 partners (partition-aligned with M chunk 16)
Mixed-agent chunks (16..23) get their c4[t,a(slot)] factor from a tiny PE
matmul (c4T [4,64] x one-hot [4,1024]) instead of static ranges. Before the
scatter, duplicates are folded with four DVE adds (M += S1, M0 += S2, M0 += S4,
H += S3), leaving unique rows only. Unused slots carry zero payload and point
at host-chosen dump rows that the call never really targets (+0.0 RMW is
harmless). The program is fully static across cores; only tensor data varies.
"""

import numpy as np

import concourse.bacc as bacc
import concourse.bass as bass
import concourse.mybir as mybir
import concourse.tile as tile
from concourse.bass_utils import run_bass_kernel_spmd

B, T, A, S = 8, 64, 4, 512
V, EXT_V = 32000, 33000
P = 128
KC = A * S  # 2048 contributions per batch element

MAIN = 32768  # out_main rows; = 256*128
HI = 256  # out_hi rows; idx = v - MAIN
SPP = MAIN // P  # 256 rows per partition stripe
VPART = V // SPP  # 125 partitions carry vocab rows
ROW_CHUNKS = [43, 43, 43, 43, 42, 42]  # c-chunks of the base stream (sum = SPP)

N_SINGLE_CH = 16  # chunks 0..15: per-agent singles (512 per agent)
M_CH = (16, 17)  # A-call merge chunks
H_CH = 18
S1_CH = (19, 20)
S2_CH = 21
S3_CH = 22
S4_CH = 23
NCH = 24
NSLOT = NCH * P  # 3072
A_CAP = 18 * P  # 2304 (chunks 0..17)
H_CAP = P
SPECIAL0 = 16 * P  # first matmul-multiplied slot

_prog = None


class _nullctx:
    def __enter__(self):
        return None

    def __exit__(self, *a):
        return False


def _build_program(loop_n=None, ablate=()):
    """loop_n: on-device repeat loop (bench variant; outputs then meaningless).
    ablate: subset of {"scatter", "base", "items", "fence", "prep"} (bench)."""
    ablate = set(ablate)
    nc = bacc.Bacc("TRN2", target_bir_lowering=False)
    f32 = mybir.dt.float32
    vocab_t = nc.dram_tensor("vocab_t", [V, T], f32, kind="ExternalInput")
    agat_t = nc.dram_tensor("agat_t", [A, T], f32, kind="ExternalInput")
    gen_t = nc.dram_tensor("gen_t", [A, T], f32, kind="ExternalInput")
    attn_slots = nc.dram_tensor("attn_slots", [T, NSLOT], f32, kind="ExternalInput")
    onehot_t = nc.dram_tensor("onehot_t", [A, NSLOT - SPECIAL0], f32, kind="ExternalInput")
    ident_in = nc.dram_tensor("ident_in", [T, T], f32, kind="ExternalInput")
    idx_a = nc.dram_tensor("idx_a", [P, A_CAP // 16], mybir.dt.int16, kind="ExternalInput")
    idx_h = nc.dram_tensor("idx_h", [P, H_CAP // 16], mybir.dt.int16, kind="ExternalInput")
    out_main = nc.dram_tensor("out_main", [MAIN, T], f32, kind="ExternalOutput")
    out_hi = nc.dram_tensor("out_hi", [HI, T], f32, kind="ExternalOutput")

    with tile.TileContext(nc) as tc:
        with (
            tc.tile_pool(name="small", bufs=1) as small,
            tc.tile_pool(name="wpool", bufs=1) as wpool,
            tc.tile_pool(name="vt", bufs=3) as vtp,
            tc.tile_pool(name="sc", bufs=3) as scp,
            tc.tile_pool(name="psum1", bufs=1, space="PSUM") as psum1,
            tc.tile_pool(name="psumc", bufs=2, space="PSUM") as psumc,
            tc.tile_pool(name="psum", bufs=4, space="PSUM") as psum,
            (tc.For_i(0, loop_n, 1) if loop_n else _nullctx()),
        ):
            # ---- per-(t,a) coefficients ----
            agat_sb = small.tile([A, T], f32)
            gen_sb = small.tile([A, T], f32)
            nc.sync.dma_start(agat_sb[:], agat_t[:])
            nc.sync.dma_start(gen_sb[:], gen_t[:])

            prod = small.tile([A, T], f32)
            nc.vector.tensor_mul(prod[:], agat_sb[:], gen_sb[:])
            ones4 = small.tile([A, P], f32)
            nc.vector.memset(ones4[:], 1.0)
            coef_ps = psum1.tile([P, T], f32, space="PSUM")
            nc.tensor.matmul(coef_ps[:], lhsT=ones4[:], rhs=prod[:], start=True, stop=True)
            coef_bc = small.tile([P, T], f32)  # coef[t] on all partitions
            nc.vector.tensor_copy(coef_bc[:], coef_ps[:])

            one4 = small.tile([A, T], f32)
            nc.vector.memset(one4[:], 1.0)
            c4t = small.tile([A, T], f32)  # c4T[a, t] = agent_attn*(1-gen)
            nc.vector.tensor_sub(c4t[:], one4[:], gen_sb[:])
            nc.vector.tensor_mul(c4t[:], c4t[:], agat_sb[:])

            # ---- small loads issued early (ahead of vocab in the qSP FIFO) ----
            attn_sb = wpool.tile([T, NSLOT], f32)
            nc.sync.dma_start(attn_sb[:], attn_slots[:])
            onehot_sb = small.tile([A, NSLOT - SPECIAL0], f32)
            nc.sync.dma_start(onehot_sb[:], onehot_t[:])
            identT = small.tile([T, T], f32)
            nc.sync.dma_start(identT[:], ident_in[:])
            ia = small.tile([P, A_CAP // 16], mybir.dt.int16)
            nc.sync.dma_start(ia[:], idx_a[:])
            ih = small.tile([P, H_CAP // 16], mybir.dt.int16)
            nc.sync.dma_start(ih[:], idx_h[:])

            # c4 [64, 4] (for tensor_scalar per-partition use)
            c4_ps = psum1.tile([T, A], f32, space="PSUM")
            nc.tensor.transpose(c4_ps[:], c4t[:], identT[:A, :A])
            c4 = small.tile([T, A], f32)
            nc.vector.tensor_copy(c4[:], c4_ps[:])

            # ---- base: out_main[v,t] = coef[t] * vocab_t[v,t] on 125 stripes ----
            if "base" not in ablate:
                vview = vocab_t[:].rearrange("(p c) t -> p c t", p=VPART, c=SPP)
                oview = out_main[0 : VPART * SPP, :].rearrange(
                    "(p c) t -> p c t", p=VPART, c=SPP
                )
                r0 = 0
                for rj in ROW_CHUNKS:
                    vt = vtp.tile([VPART, rj * T], f32, tag="vt")
                    nc.sync.dma_start(vt[:], vview[:, r0 : r0 + rj, :])
                    sc = scp.tile([VPART, rj * T], f32, tag="sc")
                    nc.vector.tensor_tensor(
                        out=sc[:].rearrange("p (c t) -> p c t", c=rj),
                        in0=vt[:].rearrange("p (c t) -> p c t", c=rj),
                        in1=coef_bc[:VPART, None, :].to_broadcast([VPART, rj, T]),
                        op=mybir.AluOpType.mult,
                    )
                    nc.scalar.dma_start(oview[:, r0 : r0 + rj, :], sc[:])
                    r0 += rj

            # ---- scatter payload w[t, slot] ----
            do_items = "items" not in ablate
            w = wpool.tile([T, NSLOT], f32)
            if do_items:
                for a in range(A):  # singles: chunks 0..15, static per-a ranges
                    nc.vector.tensor_scalar(
                        out=w[:, a * 512 : (a + 1) * 512],
                        in0=attn_sb[:, a * 512 : (a + 1) * 512],
                        scalar1=c4[:, a : a + 1],
                        scalar2=None,
                        op0=mybir.AluOpType.mult,
                    )
                # special chunks 16..23: cmul = c4T.T @ onehot via PE
                nspec = NSLOT - SPECIAL0  # 1024
                for j0 in range(0, nspec, 512):
                    j1 = min(j0 + 512, nspec)
                    cm = psumc.tile([T, j1 - j0], f32, space="PSUM", tag="cmul")
                    nc.tensor.matmul(
                        cm[:], lhsT=c4t[:], rhs=onehot_sb[:, j0:j1], start=True, stop=True
                    )
                    nc.vector.tensor_tensor(
                        out=w[:, SPECIAL0 + j0 : SPECIAL0 + j1],
                        in0=attn_sb[:, SPECIAL0 + j0 : SPECIAL0 + j1],
                        in1=cm[:],
                        op=mybir.AluOpType.mult,
                    )
                items = wpool.tile([P, NCH * T], f32)
                for ch in range(NCH):
                    tp = psum.tile([P, T], f32, space="PSUM")
                    nc.tensor.transpose(tp[:], w[:, ch * P : (ch + 1) * P], identT[:])
                    nc.vector.tensor_copy(items[:, ch * T : (ch + 1) * T], tp[:])

                # fold duplicates: M += S1, M0 += S2, M0 += S4, H += S3
                def _add(dst_ch, src_ch, n=1):
                    nc.vector.tensor_add(
                        out=items[:, dst_ch * T : (dst_ch + n) * T],
                        in0=items[:, dst_ch * T : (dst_ch + n) * T],
                        in1=items[:, src_ch * T : (src_ch + n) * T],
                    )

                _add(M_CH[0], S1_CH[0], n=2)
                _add(M_CH[0], S2_CH)
                _add(M_CH[0], S4_CH)
                _add(H_CH, S3_CH)

            # ---- scatter-add calls ----
            if "scatter" not in ablate:
                prep = "prep" in ablate
                kw = {}
                sems = []
                if prep:
                    sem_a = nc.alloc_semaphore("scat_a")
                    sem_h = nc.alloc_semaphore("scat_h")
                nc.gpsimd.dma_scatter_add(
                    out_main[:, :],
                    items[:, 0 : 18 * T].rearrange("p (c t) -> p c t", c=18),
                    ia[:],
                    A_CAP,
                    A_CAP,
                    T,
                    **({"prepare_only": True, "sem": sem_a} if prep else {}),
                )
                nc.gpsimd.dma_scatter_add(
                    out_hi[:, :],
                    items[:, H_CH * T : (H_CH + 1) * T].rearrange("p (c t) -> p c t", c=1),
                    ih[:],
                    H_CAP,
                    H_CAP,
                    T,
                    **({"prepare_only": True, "sem": sem_h} if prep else {}),
                )
                if prep:
                    nc.gpsimd.trigger_dma(count=None)

            if "fence" in ablate:
                for h, o in enumerate([out_main, out_hi]):
                    fr = small.tile([P, T], f32, tag=f"fence{h}")
                    nc.sync.dma_start(fr[:], o[0:P, :])
                    fs = small.tile([P, 1], f32, tag=f"fsum{h}")
                    nc.vector.reduce_sum(out=fs[:], in_=fr[:], axis=mybir.AxisListType.X)

    nc.compile()
    return nc


def _pack_core(vocab_b, gen_b, agat_b, attn_b, article_b):
    """Host-side data layout for one batch element (no float arithmetic)."""
    v = article_b.reshape(-1).astype(np.int64)  # contribution k = a*S + s
    a_of = np.repeat(np.arange(A), S)
    attn_flat = np.ascontiguousarray(attn_b.reshape(T, KC), dtype=np.float32)

    slots = np.zeros((T, NSLOT), np.float32)
    onehot = np.zeros((A, NSLOT - SPECIAL0), np.float32)
    tab_a = np.full(A_CAP, -1, np.int64)
    tab_h = np.full(H_CAP, -1, np.int64)

    # group contributions by destination row
    groups = {}
    for k in range(KC):
        groups.setdefault(int(v[k]), []).append(k)

    def put(ch, p, k):  # place contribution k at payload slot (partition p, chunk ch)
        slot = ch * P + p
        slots[:, slot] = attn_flat[:, k]
        if slot >= SPECIAL0:
            onehot[int(a_of[k]), slot - SPECIAL0] = 1.0

    a_cnt = np.zeros(A, np.int64)
    m_cnt = 0  # merge groups placed (over chunks 16,17 / S1 19,20)
    h_cnt = 0
    used_h = set()
    for vv, ks in groups.items():
        if vv >= MAIN:
            if len(ks) > 2:
                raise RuntimeError("hi row multiplicity > 2 unsupported")
            p = h_cnt
            h_cnt += 1
            if h_cnt > P:
                raise RuntimeError("hi capacity exceeded")
            tab_h[p] = vv - MAIN
            used_h.add(vv - MAIN)
            put(H_CH, p, ks[0])
            if len(ks) > 1:
                put(S3_CH, p, ks[1])
        elif len(ks) == 1:
            aa = int(a_of[ks[0]])
            if a_cnt[aa] >= 512:
                raise RuntimeError("singles capacity exceeded")
            pos = aa * 512 + a_cnt[aa]  # slot among chunks 0..15
            a_cnt[aa] += 1
            slots[:, pos] = attn_flat[:, ks[0]]
            tab_a[pos] = vv
        else:
            if len(ks) > 4:
                raise RuntimeError("row multiplicity > 4 unsupported")
            if len(ks) > 2:  # needs S2/S4 -> must sit in M chunk 16
                if m_cnt >= P:
                    raise RuntimeError("deep-duplicate capacity exceeded")
                p, ch_i = m_cnt, 0
            else:
                if m_cnt >= 2 * P:
                    raise RuntimeError("duplicate capacity exceeded")
                p, ch_i = m_cnt % P, m_cnt // P
            m_cnt += 1
            put(M_CH[ch_i], p, ks[0])
            put(S1_CH[ch_i], p, ks[1])
            if len(ks) > 2:
                put(S2_CH, p, ks[2])
            if len(ks) > 3:
                put(S4_CH, p, ks[3])
            tab_a[(16 + ch_i) * P + p] = vv

    # dump rows for unused A slots: rows this call never really targets
    used_a = set(tab_a[tab_a >= 0].tolist())
    free = 0
    for pos in range(A_CAP):
        if tab_a[pos] < 0:
            while free in used_a:
                free += 1
            tab_a[pos] = free
            free += 1
    free = 0
    for pos in range(H_CAP):
        if tab_h[pos] < 0:
            while free in used_h:
                free += 1
            tab_h[pos] = free
            free += 1

    def rep16(tab):  # entry k at [k%16, k//16], replicated to 128 partitions
        return np.ascontiguousarray(np.tile(tab.astype(np.int16).reshape(-1, 16).T, (8, 1)))

    return {
        "vocab_t": np.ascontiguousarray(vocab_b.T, dtype=np.float32),
        "agat_t": np.ascontiguousarray(agat_b.T, dtype=np.float32),
        "gen_t": np.ascontiguousarray(gen_b.T, dtype=np.float32),
        "attn_slots": slots,
        "onehot_t": onehot,
        "ident_in": np.eye(T, dtype=np.float32),
        "idx_a": rep16(tab_a),
        "idx_h": rep16(tab_h),
    }


def kernel(vocab_probs, generation_probs, agentwise_attn, agent_attn, article):
    global _prog
    vocab_probs = np.asarray(vocab_probs, dtype=np.float32)
    generation_probs = np.asarray(generation_probs, dtype=np.float32)
    agentwise_attn = np.asarray(agentwise_attn, dtype=np.float32)
    agent_attn = np.asarray(agent_attn, dtype=np.float32)
    article = np.asarray(article)

    if _prog is None:
        _prog = _build_program()

    in_maps = [
        _pack_core(
            vocab_probs[b], generation_probs[b], agat_b=agent_attn[b],
            attn_b=agentwise_attn[b], article_b=article[b],
        )
        for b in range(B)
    ]
    res = run_bass_kernel_spmd(_prog, in_maps, core_ids=list(range(B)))
    full = np.empty((B, T, EXT_V), np.float32)
    for b, r in enumerate(res.results):
        full[b, :, :MAIN] = r["out_main"].T
        full[b, :, MAIN:] = r["out_hi"][: EXT_V - MAIN].T
    return full



# revision 6
# speedup vs baseline: 2.5090x; 2.5090x over previous
"""Trainium2 Bass kernel for nn_MultiAgentsSummarizer (pointer-generator style
multi-agent summarizer distribution).

Math (per batch b, with T=64 target positions, A=4 agents, S=512 source tokens,
V=32000 vocab, EXT_V=33000 extended vocab):

    coef[t]   = sum_a agent_attn[t,a] * gen[t,a]
    out[t,v]  = coef[t] * vocab_probs[t,v]            (v <  V;  0 for v >= V)
    out[t, article[a,s]] += agent_attn[t,a]*(1-gen[t,a]) * agentwise_attn[t,a,s]

Strategy: one batch element per NeuronCore (B=8 = n_cores). Device work runs in
a v-major staging layout S[row, t] (fp16, x4096 scaling for fp16 range), where
the row order is a HOST-CHOSEN PERMUTATION of v. The permutation puts every
scatter-touched v (~2000 distinct rows) into a dedicated "hot" stripe: staging
row p*256 + c with c < 17 (slot (p,c), p=partition, 128*17 = 2176 slots). The
base stream processes staging rows as 128 partitions x 256 rows in c-chunks
[17, 48, 48, 48, 48, 47]; chunk 0 IS the hot stripe, so the entire scatter_add
reduces to one dense DVE add of the merged contribution tile into chunk 0's
SBUF tile before its store. No GPSIMD scatter, no RMW, no index tables on
device. All floating-point arithmetic runs on device; the host only reorders /
relabels / casts (permutation, fp16 cast with exact x4096 scale, slot packing).

Contribution payloads: slot (p,c) holds the attn vector (T values) of one
distinct touched v; its coefficient c4[t,a]*4096 is applied on-device via a
tiny PE matmul (block-diag c4 against a host one-hot of each slot's agent).
Duplicate v's (same v hit 2-4x, different agents -- product coefficients can't
merge) get mirror slots in columns 17/18/19 at the same partition (dup groups
are pinned to c=0, partition = dup ordinal), folded with 3 DVE adds before the
hot add. Rows >= V (OOV region) get zero vocab rows so the base stream yields
coef*0; untouched-hi overflow rows [32768, 33024) rely on PJRT-donated
pre-zeroed output buffers. Output unshard gathers rows by the permutation and
divides by 4096 (exact).
"""

import numpy as np

import concourse.bacc as bacc
import concourse.bass as bass
import concourse.mybir as mybir
import concourse.tile as tile
from concourse.bass_utils import run_bass_kernel_spmd

B, T, A, S = 8, 64, 4, 512
V, EXT_V = 32000, 33000
P = 128
KC = A * S  # 2048 contributions per batch element

SPP = 256  # staging rows per partition
NSTREAM = P * SPP  # 32768 streamed rows
NSTAGE = 33024  # + 256 overflow rows (untouched OOV only)
HOTC = 17  # hot columns per partition (2176 slots >= ~2000 touched rows)
NCOL = 20  # 17 hot + 3 duplicate-mirror columns
ROW_CHUNKS = [HOTC, 48, 48, 48, 48, 47]  # c-chunks (sum = SPP); chunk 0 = hot
SCALE = 4096.0  # fp16 range scaling (exact power of 2)

_prog = None


class _nullctx:
    def __enter__(self):
        return None

    def __exit__(self, *a):
        return False


def _build_program(loop_n=None, ablate=()):
    """loop_n: on-device repeat loop (bench variant; outputs then meaningless).
    ablate: subset of {"items", "hotadd", "mult", "store"} (bench variants)."""
    ablate = set(ablate)
    nc = bacc.Bacc("TRN2", target_bir_lowering=False)
    f32 = mybir.dt.float32
    f16 = mybir.dt.float16
    vocab_st = nc.dram_tensor("vocab_st", [NSTREAM, T], f16, kind="ExternalInput")
    agat_t = nc.dram_tensor("agat_t", [A, T], f32, kind="ExternalInput")
    gen_t = nc.dram_tensor("gen_t", [A, T], f32, kind="ExternalInput")
    attn_t = nc.dram_tensor("attn_t", [P, NCOL * T], f16, kind="ExternalInput")
    onehot_t = nc.dram_tensor("onehot_t", [16, 5 * P], f16, kind="ExternalInput")
    mask_t = nc.dram_tensor("mask_t", [16, 4 * T], f16, kind="ExternalInput")
    rep4_t = nc.dram_tensor("rep4_t", [A, 16], f32, kind="ExternalInput")
    out_st = nc.dram_tensor("out_st", [NSTAGE, T], f16, kind="ExternalOutput")

    do_items = "items" not in ablate
    do_hotadd = do_items and "hotadd" not in ablate
    do_mult = "mult" not in ablate
    do_store = "store" not in ablate

    with tile.TileContext(nc) as tc:
        with (
            tc.tile_pool(name="small", bufs=1) as small,
            tc.tile_pool(name="vt", bufs=3) as vtp,
            tc.tile_pool(name="sc", bufs=3) as scp,
            tc.tile_pool(name="psum1", bufs=1, space="PSUM") as psum1,
            tc.tile_pool(name="psumc", bufs=2, space="PSUM") as psumc,
            (tc.For_i(0, loop_n, 1) if loop_n else _nullctx()),
        ):
            # ---- small loads (ahead of vocab in the qSP FIFO) ----
            agat_sb = small.tile([A, T], f32)
            gen_sb = small.tile([A, T], f32)
            nc.sync.dma_start(agat_sb[:], agat_t[:])
            nc.sync.dma_start(gen_sb[:], gen_t[:])
            attn_sb = small.tile([P, NCOL * T], f16)
            nc.sync.dma_start(attn_sb[:], attn_t[:])
            onehot_sb = small.tile([16, 5 * P], f16)
            nc.sync.dma_start(onehot_sb[:], onehot_t[:])
            mask_sb = small.tile([16, 4 * T], f16)
            nc.sync.dma_start(mask_sb[:], mask_t[:])
            rep4_sb = small.tile([A, 16], f32)
            nc.sync.dma_start(rep4_sb[:], rep4_t[:])

            # ---- coefficients ----
            prod = small.tile([A, T], f32)
            nc.vector.tensor_mul(prod[:], agat_sb[:], gen_sb[:])
            ones4 = small.tile([A, P], f32)
            nc.vector.memset(ones4[:], 1.0)
            coef_ps = psum1.tile([P, T], f32, space="PSUM")
            nc.tensor.matmul(coef_ps[:], lhsT=ones4[:], rhs=prod[:], start=True, stop=True)
            coef16 = small.tile([P, T], f16)  # coef[t] on all partitions
            nc.vector.tensor_copy(coef16[:], coef_ps[:])

            c4t = small.tile([A, T], f32)  # c4T[a, t] = agent_attn*(1-gen)
            nc.vector.tensor_sub(c4t[:], agat_sb[:], prod[:])

            # rhs for per-slot coefficients: 4-chunk block-diag of c4t*SCALE.
            # rep_ps[c, t] = c4t[c%4, t] on 16 partitions (PE), then the host
            # mask (SCALE on diagonal blocks, 0 off) selects the block-diag.
            rep_ps = psum1.tile([16, T], f32, space="PSUM", tag="rep")
            nc.tensor.matmul(rep_ps[:], lhsT=rep4_sb[:], rhs=c4t[:], start=True, stop=True)
            rhs16 = small.tile([16, 4 * T], f16)
            nc.vector.tensor_tensor(
                out=rhs16[:].rearrange("p (j t) -> p j t", j=4),
                in0=mask_sb[:].rearrange("p (j t) -> p j t", j=4),
                in1=rep_ps[:, None, :].to_broadcast([16, 4, T]),
                op=mybir.AluOpType.mult,
            )

            # ---- contribution payloads: items[p, c*T+t] = attn * c4[t, a(p,c)] ----
            items = small.tile([P, NCOL * T], f16)
            if do_items:
                for g in range(5):  # 5 groups of 4 columns
                    cm = psumc.tile([P, 4 * T], f32, space="PSUM", tag="cmul")
                    nc.tensor.matmul(
                        cm[:],
                        lhsT=onehot_sb[:, g * P : (g + 1) * P],
                        rhs=rhs16[:],
                        start=True,
                        stop=True,
                    )
                    nc.vector.tensor_tensor(
                        out=items[:, g * 4 * T : (g + 1) * 4 * T],
                        in0=attn_sb[:, g * 4 * T : (g + 1) * 4 * T],
                        in1=cm[:],
                        op=mybir.AluOpType.mult,
                    )
                # fold duplicate mirrors (columns 17,18,19) into column 0
                for mc in (HOTC, HOTC + 1, HOTC + 2):
                    nc.vector.tensor_add(
                        out=items[:, 0:T],
                        in0=items[:, 0:T],
                        in1=items[:, mc * T : (mc + 1) * T],
                    )

            # ---- base stream: out[row, t] = coef[t] * vocab_st[row, t] ----
            vview = vocab_st[:].rearrange("(p c) t -> p c t", p=P, c=SPP)
            oview = out_st[0:NSTREAM, :].rearrange("(p c) t -> p c t", p=P, c=SPP)
            r0 = 0
            for ci, rj in enumerate(ROW_CHUNKS):
                vt = vtp.tile([P, rj * T], f16, tag="vt")
                nc.sync.dma_start(vt[:], vview[:, r0 : r0 + rj, :])
                src = vt
                if do_mult:
                    sc = scp.tile([P, rj * T], f16, tag="sc")
                    nc.vector.tensor_tensor(
                        out=sc[:].rearrange("p (c t) -> p c t", c=rj),
                        in0=vt[:].rearrange("p (c t) -> p c t", c=rj),
                        in1=coef16[:, None, :].to_broadcast([P, rj, T]),
                        op=mybir.AluOpType.mult,
                    )
                    if ci == 0 and do_hotadd:
                        nc.vector.tensor_add(
                            out=sc[:], in0=sc[:], in1=items[:, 0 : HOTC * T]
                        )
                    src = sc
                if do_store:
                    nc.scalar.dma_start(oview[:, r0 : r0 + rj, :], src[:])
                r0 += rj

    nc.compile()
    return nc


def _pack_core(vocab_b, gen_b, agat_b, attn_b, article_b):
    """Host-side data layout for one batch element.

    Returns (in_map, stag_row_of_v[33000]) -- all float work is relabeling,
    an exact x4096 scale, and fp16 casts; sums/products happen on device.
    """
    v = np.asarray(article_b).reshape(-1).astype(np.int64)  # k = a*S + s
    a_of = (np.arange(KC) // S).astype(np.int64)
    attn_flat = np.ascontiguousarray(
        np.asarray(attn_b).reshape(T, KC), dtype=np.float32
    )

    vals, inv, counts = np.unique(v, return_inverse=True, return_counts=True)
    G = len(vals)
    assert G <= HOTC * P, f"touched rows {G} exceed hot capacity"
    assert counts.max() <= 4, "row multiplicity > 4 unsupported"
    dup_mask = counts >= 2
    ndup = int(dup_mask.sum())
    assert ndup <= P, f"duplicate groups {ndup} exceed {P}"

    # slot ids: dup groups first (slots 0..ndup-1 -> (p=slot, c=0)), singles after
    slot_of_group = np.empty(G, np.int64)
    slot_of_group[dup_mask] = np.arange(ndup)
    slot_of_group[~dup_mask] = ndup + np.arange(G - ndup)

    # rank of each contribution within its group (stable order)
    order = np.argsort(inv, kind="stable")
    starts = np.concatenate([[0], np.cumsum(counts)])
    rank = np.empty(KC, np.int64)
    rank[order] = np.arange(KC) - starts[inv[order]]

    slot_k = slot_of_group[inv]
    part_k = slot_k % P
    col_k = np.where(rank == 0, slot_k // P, HOTC - 1 + rank)

    attnT = np.zeros((P, NCOL, T), np.float32)
    attnT[part_k, col_k, :] = attn_flat.T  # [KC, T] -> slots
    onehot = np.zeros((16, 5, P), np.float32)
    onehot[(col_k % 4) * A + a_of, col_k // 4, part_k] = 1.0

    # permutation: touched v -> hot rows; untouched fill the rest
    stag_row_of_v = np.empty(EXT_V, np.int64)
    rows_touched = (slot_of_group % P) * SPP + slot_of_group // P
    stag_row_of_v[vals] = rows_touched
    free_stream = np.setdiff1d(np.arange(NSTREAM), rows_touched, assume_unique=False)
    touched_mask = np.zeros(EXT_V, bool)
    touched_mask[vals] = True
    unt_vocab = np.nonzero(~touched_mask[:V])[0]
    unt_hi = np.nonzero(~touched_mask[V:])[0] + V
    assert len(unt_vocab) <= len(free_stream), "stream cannot hold vocab rows"
    stag_row_of_v[unt_vocab] = free_stream[: len(unt_vocab)]
    rem = free_stream[len(unt_vocab) :]
    n_hi_stream = len(rem)
    over = NSTAGE - NSTREAM
    assert len(unt_hi) <= n_hi_stream + over, "OOV overflow exceeded"
    stag_row_of_v[unt_hi[:n_hi_stream]] = rem
    stag_row_of_v[unt_hi[n_hi_stream:]] = NSTREAM + np.arange(
        len(unt_hi) - n_hi_stream
    )

    vocab_st = np.zeros((NSTREAM, T), np.float16)
    vocab_st[stag_row_of_v[:V]] = (
        np.asarray(vocab_b).T.astype(np.float32) * SCALE
    ).astype(np.float16)

    mask = np.zeros((16, 4, T), np.float16)
    for j in range(4):
        mask[4 * j : 4 * (j + 1), j, :] = SCALE
    rep4 = np.zeros((A, 16), np.float32)
    rep4[np.arange(16) % A, np.arange(16)] = 1.0

    in_map = {
        "vocab_st": vocab_st,
        "agat_t": np.ascontiguousarray(np.asarray(agat_b).T, dtype=np.float32),
        "gen_t": np.ascontiguousarray(np.asarray(gen_b).T, dtype=np.float32),
        "attn_t": attnT.reshape(P, NCOL * T).astype(np.float16),
        "onehot_t": onehot.reshape(16, 5 * P).astype(np.float16),
        "mask_t": mask.reshape(16, 4 * T),
        "rep4_t": rep4,
    }
    return in_map, stag_row_of_v


def kernel(vocab_probs, generation_probs, agentwise_attn, agent_attn, article):
    global _prog
    vocab_probs = np.asarray(vocab_probs, dtype=np.float32)
    generation_probs = np.asarray(generation_probs, dtype=np.float32)
    agentwise_attn = np.asarray(agentwise_attn, dtype=np.float32)
    agent_attn = np.asarray(agent_attn, dtype=np.float32)
    article = np.asarray(article)

    if _prog is None:
        _prog = _build_program()

    packed = [
        _pack_core(
            vocab_probs[b], generation_probs[b], agat_b=agent_attn[b],
            attn_b=agentwise_attn[b], article_b=article[b],
        )
        for b in range(B)
    ]
    in_maps = [p[0] for p in packed]
    res = run_bass_kernel_spmd(_prog, in_maps, core_ids=list(range(B)))
    full = np.empty((B, T, EXT_V), np.float32)
    inv_scale = np.float32(1.0 / SCALE)
    for b, r in enumerate(res.results):
        stag = np.asarray(r["out_st"])
        full[b] = stag[packed[b][1]].astype(np.float32).T * inv_scale
    return full


# revision 9
# speedup vs baseline: 16.7615x; 6.6807x over previous
"""Trainium2 Bass kernel for nn_MultiAgentsSummarizer (pointer-generator style
multi-agent summarizer distribution).

Math (per batch b, with T=64 target positions, A=4 agents, S=512 source tokens,
V=32000 vocab, EXT_V=33000 extended vocab):

    coef[t]   = sum_a agent_attn[t,a] * gen[t,a]
    out[t,v]  = coef[t] * vocab_probs[t,v]            (v <  V;  0 for v >= V)
    out[t, article[a,s]] += agent_attn[t,a]*(1-gen[t,a]) * agentwise_attn[t,a,s]

Strategy: one batch element per NeuronCore (B=8 = n_cores). Device work runs in
a v-major staging layout S[row, t] (fp16, x4096 scaling for fp16 range), where
the row order is a HOST-CHOSEN PERMUTATION of v. The permutation puts every
scatter-touched v (~2000 distinct rows) into a dedicated "hot" stripe: staging
row p*256 + c with c < 17 (slot (p,c), p=partition, 128*17 = 2176 slots). The
base stream processes staging rows as 128 partitions x 256 rows in c-chunks
[17, 48, 48, 48, 48, 47]; chunk 0 IS the hot stripe, so the entire scatter_add
reduces to one dense DVE add of the merged contribution tile into chunk 0's
SBUF tile before its store. No GPSIMD scatter, no RMW, no index tables on
device. All floating-point arithmetic runs on device; the host only reorders /
relabels / casts (permutation, fp16 cast with exact x4096 scale, slot packing).

Contribution payloads: slot (p,c) holds the attn vector (T values) of one
distinct touched v; its coefficient c4[t,a]*4096 is applied on-device via a
tiny PE matmul (block-diag c4 against a host one-hot of each slot's agent).
Duplicate v's (same v hit 2-4x, different agents -- product coefficients can't
merge) get mirror slots in columns 17/18/19 at the same partition (dup groups
are pinned to c=0, partition = dup ordinal), folded with 3 DVE adds before the
hot add. Rows >= V (OOV region) get zero vocab rows so the base stream yields
coef*0; untouched-hi overflow rows [32768, 33024) rely on PJRT-donated
pre-zeroed output buffers. Output unshard gathers rows by the permutation and
divides by 4096 (exact).
"""

import numpy as np

import concourse.bacc as bacc
import concourse.bass as bass
import concourse.mybir as mybir
import concourse.tile as tile
from concourse.bass_utils import run_bass_kernel_spmd

B, T, A, S = 8, 64, 4, 512
V, EXT_V = 32000, 33000
P = 128
KC = A * S  # 2048 contributions per batch element

SPP = 256  # staging rows per partition
NSTREAM = P * SPP  # 32768 staging rows
NSTAGE = 33024  # + 256 overflow rows (untouched OOV only)
HOTC = 17  # hot columns per partition (2176 slots >= ~2000 touched rows)
NCOL = 20  # 17 hot + 3 duplicate-mirror columns
CSTREAM = 251  # streamed columns; c in [251,256) holds untouched OOV (zeros)
ROW_CHUNKS = [HOTC, 47, 47, 47, 47, 46]  # c-chunks (sum = CSTREAM); 0 = hot
SCALE = 4096.0  # fp16 range scaling (exact power of 2)

_prog = None


class _nullctx:
    def __enter__(self):
        return None

    def __exit__(self, *a):
        return False


def _build_program(loop_n=None, ablate=()):
    """loop_n: on-device repeat loop (bench variant; outputs then meaningless).
    ablate: subset of {"items", "hotadd", "mult", "store"} (bench variants)."""
    ablate = set(ablate)
    nc = bacc.Bacc("TRN2", target_bir_lowering=False)
    f32 = mybir.dt.float32
    f16 = mybir.dt.float16
    vocab_st = nc.dram_tensor("vocab_st", [NSTREAM, T], f16, kind="ExternalInput")
    agat_t = nc.dram_tensor("agat_t", [A, T], f32, kind="ExternalInput")
    gen_t = nc.dram_tensor("gen_t", [A, T], f32, kind="ExternalInput")
    attn_t = nc.dram_tensor("attn_t", [P, NCOL * T], f16, kind="ExternalInput")
    onehot_t = nc.dram_tensor("onehot_t", [16, 5 * P], f16, kind="ExternalInput")
    mask_t = nc.dram_tensor("mask_t", [16, 4 * T], f16, kind="ExternalInput")
    rep4_t = nc.dram_tensor("rep4_t", [A, 16], f32, kind="ExternalInput")
    out_st = nc.dram_tensor("out_st", [NSTAGE, T], f16, kind="ExternalOutput")

    do_items = "items" not in ablate
    do_hotadd = do_items and "hotadd" not in ablate
    do_mult = "mult" not in ablate
    do_store = "store" not in ablate

    with tile.TileContext(nc) as tc:
        with (
            tc.tile_pool(name="small", bufs=1) as small,
            tc.tile_pool(name="vt", bufs=3) as vtp,
            tc.tile_pool(name="sc", bufs=3) as scp,
            tc.tile_pool(name="psum1", bufs=1, space="PSUM") as psum1,
            tc.tile_pool(name="psumc", bufs=2, space="PSUM") as psumc,
            (tc.For_i(0, loop_n, 1) if loop_n else _nullctx()),
        ):
            # ---- small loads (ahead of vocab in the qSP FIFO) ----
            agat_sb = small.tile([A, T], f32)
            gen_sb = small.tile([A, T], f32)
            nc.sync.dma_start(agat_sb[:], agat_t[:])
            nc.sync.dma_start(gen_sb[:], gen_t[:])
            attn_sb = small.tile([P, NCOL * T], f16)
            nc.sync.dma_start(attn_sb[:], attn_t[:])
            onehot_sb = small.tile([16, 5 * P], f16)
            nc.sync.dma_start(onehot_sb[:], onehot_t[:])
            mask_sb = small.tile([16, 4 * T], f16)
            nc.sync.dma_start(mask_sb[:], mask_t[:])
            rep4_sb = small.tile([A, 16], f32)
            nc.sync.dma_start(rep4_sb[:], rep4_t[:])

            # ---- coefficients ----
            prod = small.tile([A, T], f32)
            nc.vector.tensor_mul(prod[:], agat_sb[:], gen_sb[:])
            ones4 = small.tile([A, P], f32)
            nc.vector.memset(ones4[:], 1.0)
            coef_ps = psum1.tile([P, T], f32, space="PSUM")
            nc.tensor.matmul(coef_ps[:], lhsT=ones4[:], rhs=prod[:], start=True, stop=True)
            coef16 = small.tile([P, T], f16)  # coef[t] on all partitions
            nc.vector.tensor_copy(coef16[:], coef_ps[:])

            c4t = small.tile([A, T], f32)  # c4T[a, t] = agent_attn*(1-gen)
            nc.vector.tensor_sub(c4t[:], agat_sb[:], prod[:])

            # rhs for per-slot coefficients: 4-chunk block-diag of c4t*SCALE.
            # rep_ps[c, t] = c4t[c%4, t] on 16 partitions (PE), then the host
            # mask (SCALE on diagonal blocks, 0 off) selects the block-diag.
            rep_ps = psum1.tile([16, T], f32, space="PSUM", tag="rep")
            nc.tensor.matmul(rep_ps[:], lhsT=rep4_sb[:], rhs=c4t[:], start=True, stop=True)
            rhs16 = small.tile([16, 4 * T], f16)
            nc.vector.tensor_tensor(
                out=rhs16[:].rearrange("p (j t) -> p j t", j=4),
                in0=mask_sb[:].rearrange("p (j t) -> p j t", j=4),
                in1=rep_ps[:, None, :].to_broadcast([16, 4, T]),
                op=mybir.AluOpType.mult,
            )

            # ---- contribution payloads: items[p, c*T+t] = attn * c4[t, a(p,c)] ----
            items = small.tile([P, NCOL * T], f16)
            if do_items:
                for g in range(5):  # 5 groups of 4 columns
                    cm = psumc.tile([P, 4 * T], f32, space="PSUM", tag="cmul")
                    nc.tensor.matmul(
                        cm[:],
                        lhsT=onehot_sb[:, g * P : (g + 1) * P],
                        rhs=rhs16[:],
                        start=True,
                        stop=True,
                    )
                    nc.vector.tensor_tensor(
                        out=items[:, g * 4 * T : (g + 1) * 4 * T],
                        in0=attn_sb[:, g * 4 * T : (g + 1) * 4 * T],
                        in1=cm[:],
                        op=mybir.AluOpType.mult,
                    )
                # fold duplicate mirrors (columns 17,18,19) into column 0
                for mc in (HOTC, HOTC + 1, HOTC + 2):
                    nc.vector.tensor_add(
                        out=items[:, 0:T],
                        in0=items[:, 0:T],
                        in1=items[:, mc * T : (mc + 1) * T],
                    )

            # coef replicated along free dim once -> chunk multiplies are flat
            # 2D step-1 fp16 ops (DVE 2x_1p mode)
            crep = max(ROW_CHUNKS)
            coefrep = small.tile([P, crep * T], f16)
            nc.vector.tensor_copy(
                out=coefrep[:].rearrange("p (c t) -> p c t", c=crep),
                in_=coef16[:, None, :].to_broadcast([P, crep, T]),
            )

            # ---- base stream: out[row, t] = coef[t] * vocab_st[row, t] ----
            vview = vocab_st[:].rearrange("(p c) t -> p c t", p=P, c=SPP)
            oview = out_st[0:NSTREAM, :].rearrange("(p c) t -> p c t", p=P, c=SPP)
            r0 = 0
            for ci, rj in enumerate(ROW_CHUNKS):
                vt = vtp.tile([P, rj * T], f16, tag="vt")
                nc.sync.dma_start(vt[:], vview[:, r0 : r0 + rj, :])
                src = vt
                if do_mult:
                    sc = scp.tile([P, rj * T], f16, tag="sc")
                    nc.vector.tensor_tensor(
                        out=sc[:],
                        in0=vt[:],
                        in1=coefrep[:, 0 : rj * T],
                        op=mybir.AluOpType.mult,
                    )
                    if ci == 0 and do_hotadd:
                        nc.vector.tensor_add(
                            out=sc[:], in0=sc[:], in1=items[:, 0 : HOTC * T]
                        )
                    src = sc
                if do_store:
                    nc.scalar.dma_start(oview[:, r0 : r0 + rj, :], src[:])
                r0 += rj

    nc.compile()
    return nc


def _pack_core(vocab_b, gen_b, agat_b, attn_b, article_b):
    """Host-side data layout for one batch element.

    Returns (in_map, stag_row_of_v[33000]) -- all float work is relabeling,
    an exact x4096 scale, and fp16 casts; sums/products happen on device.
    """
    v = np.asarray(article_b).reshape(-1).astype(np.int64)  # k = a*S + s
    a_of = (np.arange(KC) // S).astype(np.int64)
    attn_flat = np.ascontiguousarray(
        np.asarray(attn_b).reshape(T, KC), dtype=np.float32
    )

    vals, inv, counts = np.unique(v, return_inverse=True, return_counts=True)
    G = len(vals)
    assert G <= HOTC * P, f"touched rows {G} exceed hot capacity"
    assert counts.max() <= 4, "row multiplicity > 4 unsupported"
    dup_mask = counts >= 2
    ndup = int(dup_mask.sum())
    assert ndup <= P, f"duplicate groups {ndup} exceed {P}"

    # slot ids: dup groups first (slots 0..ndup-1 -> (p=slot, c=0)), singles after
    slot_of_group = np.empty(G, np.int64)
    slot_of_group[dup_mask] = np.arange(ndup)
    slot_of_group[~dup_mask] = ndup + np.arange(G - ndup)

    # rank of each contribution within its group (stable order)
    order = np.argsort(inv, kind="stable")
    starts = np.concatenate([[0], np.cumsum(counts)])
    rank = np.empty(KC, np.int64)
    rank[order] = np.arange(KC) - starts[inv[order]]

    slot_k = slot_of_group[inv]
    part_k = slot_k % P
    col_k = np.where(rank == 0, slot_k // P, HOTC - 1 + rank)

    attnT = np.zeros((P, NCOL, T), np.float32)
    attnT[part_k, col_k, :] = attn_flat.T  # [KC, T] -> slots
    onehot = np.zeros((16, 5, P), np.float32)
    onehot[(col_k % 4) * A + a_of, col_k // 4, part_k] = 1.0

    # permutation: touched v -> hot rows; untouched fill the rest
    stag_row_of_v = np.empty(EXT_V, np.int64)
    rows_touched = (slot_of_group % P) * SPP + slot_of_group // P
    stag_row_of_v[vals] = rows_touched
    free_stream = np.setdiff1d(np.arange(NSTREAM), rows_touched, assume_unique=False)
    touched_mask = np.zeros(EXT_V, bool)
    touched_mask[vals] = True
    unt_vocab = np.nonzero(~touched_mask[:V])[0]
    unt_hi = np.nonzero(~touched_mask[V:])[0] + V
    # vocab rows must land in streamed columns (c < CSTREAM); untouched OOV
    # rows (zero data) preferentially fill never-streamed rows: the overflow
    # region and the tail columns c in [CSTREAM, SPP)
    lo_mask = (free_stream % SPP) < CSTREAM
    free_lo = free_stream[lo_mask]
    free_tail = free_stream[~lo_mask]
    assert len(unt_vocab) <= len(free_lo), "stream cannot hold vocab rows"
    stag_row_of_v[unt_vocab] = free_lo[: len(unt_vocab)]
    hi_rows = np.concatenate(
        [
            np.arange(NSTREAM, NSTAGE),
            free_tail,
            free_lo[len(unt_vocab) :],
        ]
    )
    assert len(unt_hi) <= len(hi_rows), "OOV overflow exceeded"
    stag_row_of_v[unt_hi] = hi_rows[: len(unt_hi)]

    vocab_st = np.zeros((NSTREAM, T), np.float16)
    vocab_st[stag_row_of_v[:V]] = (
        np.asarray(vocab_b).T.astype(np.float32) * SCALE
    ).astype(np.float16)

    mask = np.zeros((16, 4, T), np.float16)
    for j in range(4):
        mask[4 * j : 4 * (j + 1), j, :] = SCALE
    rep4 = np.zeros((A, 16), np.float32)
    rep4[np.arange(16) % A, np.arange(16)] = 1.0

    in_map = {
        "vocab_st": vocab_st,
        "agat_t": np.ascontiguousarray(np.asarray(agat_b).T, dtype=np.float32),
        "gen_t": np.ascontiguousarray(np.asarray(gen_b).T, dtype=np.float32),
        "attn_t": attnT.reshape(P, NCOL * T).astype(np.float16),
        "onehot_t": onehot.reshape(16, 5 * P).astype(np.float16),
        "mask_t": mask.reshape(16, 4 * T),
        "rep4_t": rep4,
    }
    return in_map, stag_row_of_v


def kernel(vocab_probs, generation_probs, agentwise_attn, agent_attn, article):
    global _prog
    vocab_probs = np.asarray(vocab_probs, dtype=np.float32)
    generation_probs = np.asarray(generation_probs, dtype=np.float32)
    agentwise_attn = np.asarray(agentwise_attn, dtype=np.float32)
    agent_attn = np.asarray(agent_attn, dtype=np.float32)
    article = np.asarray(article)

    if _prog is None:
        _prog = _build_program()

    packed = [
        _pack_core(
            vocab_probs[b], generation_probs[b], agat_b=agent_attn[b],
            attn_b=agentwise_attn[b], article_b=article[b],
        )
        for b in range(B)
    ]
    in_maps = [p[0] for p in packed]
    res = run_bass_kernel_spmd(_prog, in_maps, core_ids=list(range(B)))
    full = np.empty((B, T, EXT_V), np.float32)
    inv_scale = np.float32(1.0 / SCALE)
    for b, r in enumerate(res.results):
        stag = np.asarray(r["out_st"])
        full[b] = stag[packed[b][1]].astype(np.float32).T * inv_scale
    return full
